# revision 1
# baseline (speedup 1.0000x reference)
"""CapsuleLayer dynamic-routing kernel for 8 Trainium2 NeuronCores, v2.

Data-parallel over batch (8 batches/core). Layout: partition p = b*16 + i16,
i = blk*16 + i16, NBLK = 128 blocks of 16 input capsules.

  - Build: u_hat via single-pass matmuls, k = (i16, d8) = 128 contraction,
    lhsT = block-diagonal u (streamed with W), rhs = W-block [128, 512].
    s1 (uniform-c weighted sum) folded in via dense [128, 8] u lhsT.
  - Routing iters 2,3: agreement on DVE (mul + contiguous e-halves tree,
    tails split with GPSIMD), softmax batched on ACT with a 2-chunk-lagged
    Z-reduce (avoids DVE head-of-line stalls), c placed into quarter-width
    block-diagonal CM lhsT via masked broadcast-multiplies, s-pass on PE
    with j-quarter masked diag-extraction, slab-interleaved so the s-pass
    streams while agreement continues. Iter-3 logits recomputed from
    v1+v2 by linearity (no logit accumulation chain).
"""

import sys

sys.path.insert(0, "/opt/trn_rl_repo")

import numpy as np
import ml_dtypes

B, NI, DI, NO, DO = 64, 2048, 8, 32, 16
NC_CORES = 8
BL = B // NC_CORES            # 8 batches per core
JE = NO * DO                  # 512
NBLK = NI // 16               # 128 blocks of 16 input capsules
EPS = 1e-7
BF16 = ml_dtypes.bfloat16
F8 = ml_dtypes.float8_e4m3fn

_cache = {}


def _build_program():
    import concourse.bass as bass
    import concourse.bacc as bacc
    import concourse.mybir as mybir
    import concourse.tile as tile

    f32 = mybir.dt.float32
    bf16 = mybir.dt.bfloat16
    f8 = mybir.dt.float8e4

    nc = bacc.Bacc("TRN2", target_bir_lowering=False, debug=False,
                   num_devices=NC_CORES)

    GB = 4                     # blocks per W-DMA group
    NG = NBLK // GB            # 32 groups
    CH = 8                     # blocks per agreement chunk
    NCH = NBLK // CH           # 16 chunks

    # DRAM I/O (per core)
    # wu: W-block (512 cols) + u-blockdiag (128 cols) per blk
    wu_d = nc.dram_tensor("wu", [128, NBLK, 648], bf16, kind="ExternalInput")
    dm_d = nc.dram_tensor("diagmask", [64, 128], bf16, kind="ExternalInput")
    bm_d = nc.dram_tensor("bmask", [128, 64], bf16, kind="ExternalInput")
    sq_d = nc.dram_tensor("sel8", [64, BL], f32, kind="ExternalInput")
    vout_d = nc.dram_tensor("v_out", [BL, JE], f32, kind="ExternalOutput")

    with tile.TileContext(nc) as tc:
        with (
            tc.tile_pool(name="singles", bufs=1) as singles,
            tc.tile_pool(name="wstream", bufs=4) as wpool,
            tc.tile_pool(name="ppool", bufs=1) as ppool,
            tc.tile_pool(name="tpool", bufs=1) as tpool,
            tc.tile_pool(name="spool", bufs=1) as spool,
            tc.tile_pool(name="vpool", bufs=1) as vpool,
            tc.tile_pool(name="build_ps", bufs=2, space="PSUM") as build_ps,
            tc.tile_pool(name="s1_ps", bufs=1, space="PSUM") as s1_ps_pool,
            tc.tile_pool(name="sp_ps", bufs=1, space="PSUM") as sp_ps,
            tc.tile_pool(name="s_ps", bufs=1, space="PSUM") as s_ps_pool,
        ):
            # ---- persistent SBUF state ----
            UH = singles.tile([128, NBLK, JE], bf16)       # 128 KiB/part
            LOG = singles.tile([128, NBLK, NO], bf16)      # 8 KiB/part
            CM = singles.tile([128, NBLK, 64], bf16)       # 16 KiB/part
            EXN = singles.tile([128, NBLK, NO], bf16)      # 8 KiB/part
            Z = singles.tile([128, NBLK], f32)
            RZB = singles.tile([128, NBLK], bf16)
            DM = singles.tile([64, 128], bf16)
            BM = singles.tile([128, 64], bf16)
            SQ = singles.tile([64, BL], f32)
            s_sb = singles.tile([BL, JE], f32)
            vb_sb = singles.tile([BL, JE], bf16)

            nc.sync.dma_start(out=DM[:, :], in_=dm_d[:, :])
            nc.sync.dma_start(out=BM[:, :], in_=bm_d[:, :])
            nc.sync.dma_start(out=SQ[:, :], in_=sq_d[:, :])

            # ---- phase 1: build u_hat + fold s1 ----
            s1p = s1_ps_pool.tile([BL, JE], f32, tag="s1ps")
            for g in range(NG):
                wt = wpool.tile([128, GB, 648], bf16, tag="w")
                nc.sync.dma_start(out=wt[:, :, :],
                                  in_=wu_d[:, g * GB:(g + 1) * GB, :])
                for k in range(GB):
                    blk = g * GB + k
                    ps = build_ps.tile([128, JE], f32, tag="bps")
                    nc.tensor.matmul(ps[:, :], wt[:, k, 512:640],
                                     wt[:, k, 0:512], start=True, stop=True)
                    nc.tensor.matmul(s1p[:, :], wt[:, k, 640:648],
                                     wt[:, k, 0:512],
                                     start=(blk == 0), stop=(blk == NBLK - 1))
                    if blk % 2 == 0:
                        nc.vector.tensor_copy(UH[:, blk, :], ps[:, :])
                    else:
                        nc.scalar.copy(UH[:, blk, :], ps[:, :])

            # ---- squash helper: reads s_sb, writes vb_sb (t<3) or output ----
            def squash(t):
                SQT = spool.tile([BL, JE], f32, tag="SQT")
                nc.vector.tensor_mul(SQT[:, :], s_sb[:, :], s_sb[:, :])
                N2 = spool.tile([BL, NO], f32, tag="N2")
                nc.vector.tensor_reduce(
                    out=N2[:, :],
                    in_=SQT.rearrange("p (j e) -> p j e", e=DO),
                    axis=mybir.AxisListType.X,
                    op=mybir.AluOpType.add,
                )
                NE = spool.tile([BL, NO], f32, tag="NE")
                nc.vector.tensor_scalar_add(NE[:, :], N2[:, :], EPS)
                SRT = spool.tile([BL, NO], f32, tag="SRT")
                nc.scalar.activation(SRT[:, :], NE[:, :],
                                     mybir.ActivationFunctionType.Sqrt)
                T1 = spool.tile([BL, NO], f32, tag="T1q")
                nc.vector.tensor_scalar_add(T1[:, :], N2[:, :], 1.0)
                T2 = spool.tile([BL, NO], f32, tag="T2q")
                nc.vector.tensor_mul(T2[:, :], T1[:, :], SRT[:, :])
                RC = spool.tile([BL, NO], f32, tag="RCq")
                nc.vector.reciprocal(RC[:, :], T2[:, :])
                F = spool.tile([BL, NO], f32, tag="Fq")
                nc.vector.tensor_mul(F[:, :], N2[:, :], RC[:, :])
                fb = F.unsqueeze(2).broadcast_to([BL, NO, DO])
                if t < 3:
                    nc.vector.tensor_mul(
                        vb_sb.rearrange("p (j e) -> p j e", e=DO),
                        s_sb.rearrange("p (j e) -> p j e", e=DO), fb)
                else:
                    nc.vector.tensor_mul(
                        s_sb.rearrange("p (j e) -> p j e", e=DO),
                        s_sb.rearrange("p (j e) -> p j e", e=DO), fb)
                    nc.sync.dma_start(out=vout_d[:, :], in_=s_sb[:, :])

            # iter 1: s = s1 / NO, v1 = squash(s)
            VP = singles.tile([BL, JE], bf16)
            nc.vector.tensor_scalar_mul(s_sb[:, :], s1p[:, :], 1.0 / NO)
            squash(1)
            nc.vector.tensor_copy(VP[:, :], vb_sb[:, :])

            # ---- routing iterations 2, 3 ----
            for t in (2, 3):
                if t == 3:
                    # logits_3 = UH . (v1 + v2), by linearity
                    nc.vector.tensor_add(vb_sb[:, :], vb_sb[:, :], VP[:, :])
                # replicate v across partitions: VREP[b*16+i16, je] = v[b, je]
                VREP = vpool.tile([128, JE], bf16, tag="VREP")
                vv = vb_sb[:, :]
                src = bass.AP(
                    tensor=vv.tensor,
                    offset=vv.offset,
                    ap=[list(vv.ap[0]), [0, 16], list(vv.ap[1])],
                )
                nc.sync.dma_start(out=VREP[:, :], in_=src)

                # slab machinery: softmax + CM + s-pass for 32-block
                # slabs, interleaved into the agreement chunk loop so PE
                # streams the s-pass while DVE continues agreement
                s_pst = s_ps_pool.tile([BL, JE], f32, tag="sps")
                spq0 = sp_ps.tile([64, 128], f32, tag="spq0")
                spq1 = sp_ps.tile([64, 128], f32, tag="spq1")
                spq2 = sp_ps.tile([64, 128], f32, tag="spq2")
                spq3 = sp_ps.tile([64, 128], f32, tag="spq3")
                spqs = [spq0, spq1, spq2, spq3]
                bmv = (BM.rearrange("p (b j) -> p b j", j=8)
                       .unsqueeze(1).broadcast_to([128, 32, BL, 8]))
                slab_state = [0]

                def process_slab(t):
                    sl = slab_state[0]
                    slab_state[0] += 1
                    blks = slice(32 * sl, 32 * (sl + 1))
                    nc.vector.reciprocal(Z[:, blks], Z[:, blks])
                    nc.vector.tensor_copy(RZB[:, blks], Z[:, blks])
                    rzb = (RZB[:, blks].unsqueeze(2)
                           .broadcast_to([128, 32, NO]))
                    nc.vector.tensor_mul(EXN[:, blks, :], EXN[:, blks, :],
                                         rzb)
                    for q in range(4):
                        exv = (EXN[:, blks, 8 * q:8 * q + 8]
                               .unsqueeze(2)
                               .broadcast_to([128, 32, BL, 8]))
                        nc.vector.tensor_mul(
                            CM.rearrange("p k (b j) -> p k b j", j=8)
                            [:, blks, :, :], exv, bmv)
                        for blk in range(32 * sl, 32 * (sl + 1)):
                            nc.tensor.matmul(
                                spqs[q][:, :], CM[:, blk, :],
                                UH[:, blk, 128 * q:128 * (q + 1)],
                                start=(blk == 0), stop=(blk == NBLK - 1),
                                skip_group_check=True)

                # agreement: LOG (+)= sum_e UH * VREP
                vrb = VREP.unsqueeze(1).broadcast_to([128, CH, JE])
                for ch in range(NCH):
                    blks = slice(ch * CH, (ch + 1) * CH)
                    P = ppool.tile([128, CH, JE], bf16, tag="P")
                    nc.vector.tensor_mul(P[:, :, :], UH[:, blks, :], vrb)
                    Pv = P.rearrange("p c (j h e) -> p c j h e", h=2, e=8)
                    T1 = tpool.tile([128, CH, NO, 8], bf16, tag="T1")
                    nc.vector.tensor_add(T1[:, :, :, :], Pv[:, :, :, 0, :],
                                         Pv[:, :, :, 1, :])
                    T1v = T1.rearrange("p c j (h e) -> p c j h e", h=2)
                    T2 = tpool.tile([128, CH, NO, 4], bf16, tag="T2")
                    t2eng = nc.gpsimd if ch % 2 == 1 else nc.vector
                    t2eng.tensor_add(T2[:, :, :, :], T1v[:, :, :, 0, :],
                                     T1v[:, :, :, 1, :])
                    T2v = T2.rearrange("p c j (h e) -> p c j h e", h=2)
                    T3 = tpool.tile([128, CH, NO, 2], bf16, tag="T3")
                    nc.gpsimd.tensor_add(T3[:, :, :, :], T2v[:, :, :, 0, :],
                                         T2v[:, :, :, 1, :])
                    nc.gpsimd.tensor_add(LOG[:, blks, :], T3[:, :, :, 0],
                                         T3[:, :, :, 1])
                    # softmax partials; Z lags 2 chunks so the DVE wait
                    # queue head never blocks on ACT's exp
                    nc.scalar.activation(EXN[:, blks, :], LOG[:, blks, :],
                                         mybir.ActivationFunctionType.Exp)
                    if ch >= 2:
                        zb = slice((ch - 2) * CH, (ch - 1) * CH)
                        nc.vector.tensor_reduce(
                            out=Z[:, zb], in_=EXN[:, zb, :],
                            axis=mybir.AxisListType.X, op=mybir.AluOpType.add)
                    # slab sl covers chunks 4sl..4sl+3; ready after Z(4sl+3)
                    # lands, i.e. after ch == 4sl+5
                    if ch >= 5 and (ch - 5) % 4 == 0 and slab_state[0] < 3:
                        process_slab(t)
                for ch in (NCH - 2, NCH - 1):
                    blks = slice(ch * CH, (ch + 1) * CH)
                    nc.vector.tensor_reduce(
                        out=Z[:, blks], in_=EXN[:, blks, :],
                        axis=mybir.AxisListType.X, op=mybir.AluOpType.add)
                process_slab(t)
                for q in range(4):
                    ME = spool.tile([64, 128], f32, tag=f"ME{q}")
                    nc.vector.tensor_mul(ME[:, :], spqs[q][:, :], DM[:, :])
                    nc.tensor.matmul(s_pst[:, 128 * q:128 * (q + 1)],
                                     SQ[:, :], ME[:, :],
                                     start=True, stop=True)
                nc.vector.tensor_copy(s_sb[:, :], s_pst[:, :])
                squash(t)

    nc.compile()
    return nc


def _host_prep(u, W):
    """Prepack per-core operands."""
    # W-pack: w[p=(i16*8+d), blk, j*16+e] = W[blk*16+i16, j, d, e]
    w = (
        W.reshape(NBLK, 16, NO, DI, DO)          # blk, i16, j, d, e
        .transpose(1, 3, 0, 2, 4)                # i16, d, blk, j, e
        .reshape(128, NBLK, JE)
        .astype(BF16)
    )
    # u block-diag: ubd[c][p=(i16,d), blk, b*16+i16'] = u[c*8+b, blk*16+i16, d] delta
    ur = u.reshape(NC_CORES, BL, NBLK, 16, DI)   # c, b, blk, i16, d
    ubd = np.zeros((NC_CORES, 16, DI, NBLK, BL, 16), dtype=BF16)
    for i16 in range(16):
        # target rows (i16, d), cols (b, i16)
        ubd[:, i16, :, :, :, i16] = ur[:, :, :, i16, :].transpose(0, 3, 2, 1)
    ubd = ubd.reshape(NC_CORES, 128, NBLK, 128)
    # us[c][p=(i16,d), blk, b] = u[c*8+b, blk*16+i16, d]
    us = np.ascontiguousarray(
        ur.transpose(0, 3, 4, 2, 1)).reshape(NC_CORES, 128, NBLK, BL)
    us = us.astype(BF16)
    wu = np.concatenate(
        [np.broadcast_to(w[None], (NC_CORES,) + w.shape), ubd, us], axis=3)
    # diag mask dm[(b,j8), (j8',e)] = (j8 == j8')
    j8 = np.arange(64) % 8
    dm = (np.arange(128) // DO == j8[:, None]).astype(BF16)
    # bmask bm[p=(b,i16), (b',j8)] = (b == b')
    bm = (np.arange(128)[:, None] // 16 == np.arange(64)[None, :] // 8
          ).astype(BF16)
    # select sq[(b,j8), b'] = (b == b') / 8  (undo the W*8 prescale)
    sq = (np.arange(BL)[None, :] == (np.arange(64) // 8)[:, None]).astype(
        np.float32)
    return wu, dm, bm, sq


def kernel(u, W):
    from concourse.bass_utils import run_bass_kernel_spmd

    key = "prog"
    if key not in _cache:
        _cache[key] = _build_program()
    nc = _cache[key]

    wu, dm, bm, sq = _host_prep(np.asarray(u, np.float32),
                                np.asarray(W, np.float32))
    in_maps = [
        {"wu": wu[c], "diagmask": dm, "bmask": bm, "sel8": sq}
        for c in range(NC_CORES)
    ]
    res = run_bass_kernel_spmd(nc, in_maps, list(range(NC_CORES)))
    out = np.concatenate([res.results[c]["v_out"] for c in range(NC_CORES)],
                         axis=0)
    return out.reshape(B, NO, DO).astype(np.float32)



# revision 2
# speedup vs baseline: 1.0198x; 1.0198x over previous
"""CapsuleLayer dynamic-routing kernel for 8 Trainium2 NeuronCores, v3.

Data-parallel over batch (8 batches/core). Layout: partition p = b*16 + i16,
i = blk*16 + i16, NBLK = 128 blocks of 16 input capsules.

  - Build: u_hat via single-pass matmuls, k = (i16, d8) = 128 contraction,
    lhsT = block-diagonal u built ON-CHIP on GPSIMD from the dense 8-col u
    stream (cuts the W DMA stream from 648 to 520 cols), rhs = W-block
    [128, 512]. s1 (uniform-c weighted sum) folded in via dense [128, 8]
    u lhsT.
  - Routing iters 2,3: agreement on DVE (mul + contiguous e-halves tree,
    tails split with GPSIMD), softmax batched on ACT with a 2-chunk-lagged
    Z-reduce, c placed into quarter-width block-diagonal CM lhsT via masked
    broadcast-multiplies, s-pass on PE with SWAPPED operands (UH slice
    stationary, 64-wide CM moving: half the streamed columns), s extracted
    via diag-mask + j8'-reduce + PE transpose. Slab-interleaved so the
    s-pass streams while agreement continues. Iter-3 logits recomputed from
    v1+v2 by linearity.
"""

import sys

sys.path.insert(0, "/opt/trn_rl_repo")

import numpy as np
import ml_dtypes

B, NI, DI, NO, DO = 64, 2048, 8, 32, 16
NC_CORES = 8
BL = B // NC_CORES            # 8 batches per core
JE = NO * DO                  # 512
NBLK = NI // 16               # 128 blocks of 16 input capsules
EPS = 1e-7
BF16 = ml_dtypes.bfloat16
WCOLS = 520                   # 512 W + 8 dense-u

_cache = {}


def _build_program():
    import concourse.bass as bass
    import concourse.bacc as bacc
    import concourse.mybir as mybir
    import concourse.tile as tile

    f32 = mybir.dt.float32
    bf16 = mybir.dt.bfloat16

    nc = bacc.Bacc("TRN2", target_bir_lowering=False, debug=False,
                   num_devices=NC_CORES)

    GB = 4                     # blocks per W-DMA group
    NG = NBLK // GB            # 32 groups
    CH = 8                     # blocks per agreement chunk
    NCH = NBLK // CH           # 16 chunks

    # DRAM I/O (per core)
    wu_d = nc.dram_tensor("wu", [128, NBLK, WCOLS], bf16, kind="ExternalInput")
    dm_d = nc.dram_tensor("diagmask", [128, 64], bf16, kind="ExternalInput")
    bm_d = nc.dram_tensor("bmask", [128, 64], bf16, kind="ExternalInput")
    mk_d = nc.dram_tensor("imask", [128, 16], bf16, kind="ExternalInput")
    id_d = nc.dram_tensor("ident", [128, 128], f32, kind="ExternalInput")
    vout_d = nc.dram_tensor("v_out", [BL, JE], f32, kind="ExternalOutput")

    with tile.TileContext(nc) as tc:
        with (
            tc.tile_pool(name="singles", bufs=1) as singles,
            tc.tile_pool(name="wstream", bufs=4) as wpool,
            tc.tile_pool(name="ubd", bufs=3) as upool,
            tc.tile_pool(name="ppool", bufs=1) as ppool,
            tc.tile_pool(name="tpool", bufs=1) as tpool,
            tc.tile_pool(name="spool", bufs=1) as spool,
            tc.tile_pool(name="vpool", bufs=1) as vpool,
            tc.tile_pool(name="build_ps", bufs=2, space="PSUM") as build_ps,
            tc.tile_pool(name="s1_ps", bufs=1, space="PSUM") as s1_ps_pool,
            tc.tile_pool(name="sp_ps", bufs=1, space="PSUM") as sp_ps,
            tc.tile_pool(name="s_ps", bufs=1, space="PSUM") as s_ps_pool,
        ):
            # ---- persistent SBUF state ----
            UH = singles.tile([128, NBLK, JE], bf16)       # 128 KiB/part
            LOG = singles.tile([128, NBLK, NO], bf16)      # 8 KiB/part
            CM = singles.tile([128, NBLK, 64], bf16)       # 16 KiB/part
            EXN = singles.tile([128, NBLK, NO], bf16)      # 8 KiB/part
            Z = singles.tile([128, NBLK], f32)
            RZB = singles.tile([128, NBLK], bf16)
            DM = singles.tile([128, 64], bf16)
            BM = singles.tile([128, 64], bf16)
            MK = singles.tile([128, 16], bf16)
            ID = singles.tile([128, 128], f32)
            s_sb = singles.tile([BL, JE], f32)
            vb_sb = singles.tile([BL, JE], bf16)

            nc.sync.dma_start(out=DM[:, :], in_=dm_d[:, :])
            nc.sync.dma_start(out=BM[:, :], in_=bm_d[:, :])
            nc.sync.dma_start(out=MK[:, :], in_=mk_d[:, :])
            nc.sync.dma_start(out=ID[:, :], in_=id_d[:, :])

            # ---- phase 1: build u_hat + fold s1 ----
            s1p = s1_ps_pool.tile([BL, JE], f32, tag="s1ps")
            mkb = MK.unsqueeze(1).unsqueeze(2).broadcast_to([128, GB, BL, 16])
            for g in range(NG):
                wt = wpool.tile([128, GB, WCOLS], bf16, tag="w")
                nc.sync.dma_start(out=wt[:, :, :],
                                  in_=wu_d[:, g * GB:(g + 1) * GB, :])
                # block-diagonal u lhsT, built on GPSIMD: ub[p,(b,i16')] =
                # us[p,b] * (i16' == i16(p))
                ub = upool.tile([128, GB, BL, 16], bf16, tag="ub")
                usv = (wt.rearrange("p k c -> p k c")[:, :, 512:520]
                       .unsqueeze(3).broadcast_to([128, GB, BL, 16]))
                nc.gpsimd.tensor_mul(ub[:, :, :, :], usv, mkb)
                ubf = ub.rearrange("p k b i -> p k (b i)")
                for k in range(GB):
                    blk = g * GB + k
                    ps = build_ps.tile([128, JE], f32, tag="bps")
                    nc.tensor.matmul(ps[:, :], ubf[:, k, :],
                                     wt[:, k, 0:512], start=True, stop=True)
                    nc.tensor.matmul(s1p[:, :], wt[:, k, 512:520],
                                     wt[:, k, 0:512],
                                     start=(blk == 0), stop=(blk == NBLK - 1))
                    if blk % 2 == 0:
                        nc.vector.tensor_copy(UH[:, blk, :], ps[:, :])
                    else:
                        nc.scalar.copy(UH[:, blk, :], ps[:, :])

            # ---- squash helper: reads s_sb, writes vb_sb (t<3) or output ----
            def squash(t):
                SQT = spool.tile([BL, JE], f32, tag="SQT")
                nc.vector.tensor_mul(SQT[:, :], s_sb[:, :], s_sb[:, :])
                N2 = spool.tile([BL, NO], f32, tag="N2")
                nc.vector.tensor_reduce(
                    out=N2[:, :],
                    in_=SQT.rearrange("p (j e) -> p j e", e=DO),
                    axis=mybir.AxisListType.X,
                    op=mybir.AluOpType.add,
                )
                NE = spool.tile([BL, NO], f32, tag="NE")
                nc.vector.tensor_scalar_add(NE[:, :], N2[:, :], EPS)
                SRT = spool.tile([BL, NO], f32, tag="SRT")
                nc.scalar.activation(SRT[:, :], NE[:, :],
                                     mybir.ActivationFunctionType.Sqrt)
                T1 = spool.tile([BL, NO], f32, tag="T1q")
                nc.vector.tensor_scalar_add(T1[:, :], N2[:, :], 1.0)
                T2 = spool.tile([BL, NO], f32, tag="T2q")
                nc.vector.tensor_mul(T2[:, :], T1[:, :], SRT[:, :])
                RC = spool.tile([BL, NO], f32, tag="RCq")
                nc.vector.reciprocal(RC[:, :], T2[:, :])
                F = spool.tile([BL, NO], f32, tag="Fq")
                nc.vector.tensor_mul(F[:, :], N2[:, :], RC[:, :])
                fb = F.unsqueeze(2).broadcast_to([BL, NO, DO])
                if t < 3:
                    nc.vector.tensor_mul(
                        vb_sb.rearrange("p (j e) -> p j e", e=DO),
                        s_sb.rearrange("p (j e) -> p j e", e=DO), fb)
                else:
                    nc.vector.tensor_mul(
                        s_sb.rearrange("p (j e) -> p j e", e=DO),
                        s_sb.rearrange("p (j e) -> p j e", e=DO), fb)
                    nc.sync.dma_start(out=vout_d[:, :], in_=s_sb[:, :])

            # iter 1: s = s1 / NO, v1 = squash(s)
            VP = singles.tile([BL, JE], bf16)
            nc.vector.tensor_scalar_mul(s_sb[:, :], s1p[:, :], 1.0 / NO)
            squash(1)
            nc.vector.tensor_copy(VP[:, :], vb_sb[:, :])

            # ---- routing iterations 2, 3 ----
            for t in (2, 3):
                if t == 3:
                    # logits_3 = UH . (v1 + v2), by linearity
                    nc.vector.tensor_add(vb_sb[:, :], vb_sb[:, :], VP[:, :])
                # replicate v across partitions: VREP[b*16+i16, je] = v[b, je]
                VREP = vpool.tile([128, JE], bf16, tag="VREP")
                vv = vb_sb[:, :]
                src = bass.AP(
                    tensor=vv.tensor,
                    offset=vv.offset,
                    ap=[list(vv.ap[0]), [0, 16], list(vv.ap[1])],
                )
                nc.sync.dma_start(out=VREP[:, :], in_=src)

                # slab machinery: softmax + CM + s-pass for 32-block
                # slabs, interleaved into the agreement chunk loop so PE
                # streams the s-pass while DVE continues agreement
                s_pst = s_ps_pool.tile([BL, JE], f32, tag="sps")
                spq0 = sp_ps.tile([128, 64], f32, tag="spq0")
                spq1 = sp_ps.tile([128, 64], f32, tag="spq1")
                spq2 = sp_ps.tile([128, 64], f32, tag="spq2")
                spq3 = sp_ps.tile([128, 64], f32, tag="spq3")
                spqs = [spq0, spq1, spq2, spq3]
                bmv = (BM.rearrange("p (b j) -> p b j", j=8)
                       .unsqueeze(1).broadcast_to([128, 32, BL, 8]))
                slab_state = [0]

                def process_slab(t):
                    sl = slab_state[0]
                    slab_state[0] += 1
                    blks = slice(32 * sl, 32 * (sl + 1))
                    nc.vector.reciprocal(Z[:, blks], Z[:, blks])
                    nc.vector.tensor_copy(RZB[:, blks], Z[:, blks])
                    rzb = (RZB[:, blks].unsqueeze(2)
                           .broadcast_to([128, 32, NO]))
                    nc.vector.tensor_mul(EXN[:, blks, :], EXN[:, blks, :],
                                         rzb)
                    for q in range(4):
                        exv = (EXN[:, blks, 8 * q:8 * q + 8]
                               .unsqueeze(2)
                               .broadcast_to([128, 32, BL, 8]))
                        nc.vector.tensor_mul(
                            CM.rearrange("p k (b j) -> p k b j", j=8)
                            [:, blks, :, :], exv, bmv)
                        for blk in range(32 * sl, 32 * (sl + 1)):
                            nc.tensor.matmul(
                                spqs[q][:, :],
                                UH[:, blk, 128 * q:128 * (q + 1)],
                                CM[:, blk, :],
                                start=(blk == 0), stop=(blk == NBLK - 1),
                                skip_group_check=True)

                # agreement: LOG (+)= sum_e UH * VREP
                vrb = VREP.unsqueeze(1).broadcast_to([128, CH, JE])
                for ch in range(NCH):
                    blks = slice(ch * CH, (ch + 1) * CH)
                    P = ppool.tile([128, CH, JE], bf16, tag="P")
                    nc.vector.tensor_mul(P[:, :, :], UH[:, blks, :], vrb)
                    Pv = P.rearrange("p c (j h e) -> p c j h e", h=2, e=8)
                    T1 = tpool.tile([128, CH, NO, 8], bf16, tag="T1")
                    nc.vector.tensor_add(T1[:, :, :, :], Pv[:, :, :, 0, :],
                                         Pv[:, :, :, 1, :])
                    T1v = T1.rearrange("p c j (h e) -> p c j h e", h=2)
                    T2 = tpool.tile([128, CH, NO, 4], bf16, tag="T2")
                    t2eng = nc.gpsimd if ch % 2 == 1 else nc.vector
                    t2eng.tensor_add(T2[:, :, :, :], T1v[:, :, :, 0, :],
                                     T1v[:, :, :, 1, :])
                    T2v = T2.rearrange("p c j (h e) -> p c j h e", h=2)
                    T3 = tpool.tile([128, CH, NO, 2], bf16, tag="T3")
                    nc.gpsimd.tensor_add(T3[:, :, :, :], T2v[:, :, :, 0, :],
                                         T2v[:, :, :, 1, :])
                    nc.gpsimd.tensor_add(LOG[:, blks, :], T3[:, :, :, 0],
                                         T3[:, :, :, 1])
                    # softmax partials; Z lags 2 chunks so the DVE wait
                    # queue head never blocks on ACT's exp
                    nc.scalar.activation(EXN[:, blks, :], LOG[:, blks, :],
                                         mybir.ActivationFunctionType.Exp)
                    if ch >= 2:
                        zb = slice((ch - 2) * CH, (ch - 1) * CH)
                        nc.vector.tensor_reduce(
                            out=Z[:, zb], in_=EXN[:, zb, :],
                            axis=mybir.AxisListType.X, op=mybir.AluOpType.add)
                    # slab sl covers chunks 4sl..4sl+3; ready after Z(4sl+3)
                    # lands, i.e. after ch == 4sl+5
                    if ch >= 5 and (ch - 5) % 4 == 0 and slab_state[0] < 3:
                        process_slab(t)
                for ch in (NCH - 2, NCH - 1):
                    blks = slice(ch * CH, (ch + 1) * CH)
                    nc.vector.tensor_reduce(
                        out=Z[:, blks], in_=EXN[:, blks, :],
                        axis=mybir.AxisListType.X, op=mybir.AluOpType.add)
                process_slab(t)
                # s extraction: diag-mask the [(j8,e),(b,j8')] cross sums,
                # reduce over j8', transpose [128, 8] -> [8, 128] on PE
                for q in range(4):
                    ME = spool.tile([128, 64], f32, tag=f"ME{q}")
                    nc.vector.tensor_mul(ME[:, :], spqs[q][:, :], DM[:, :])
                    SR = spool.tile([128, 8], f32, tag=f"SR{q}")
                    nc.vector.tensor_reduce(
                        out=SR[:, :],
                        in_=ME.rearrange("p (b j) -> p b j", j=8),
                        axis=mybir.AxisListType.X, op=mybir.AluOpType.add)
                    nc.tensor.transpose(s_pst[:, 128 * q:128 * (q + 1)],
                                        SR[:, :], ID[:, :])
                nc.vector.tensor_copy(s_sb[:, :], s_pst[:, :])
                squash(t)

    nc.compile()
    return nc


def _host_prep(u, W):
    """Prepack per-core operands."""
    # W-pack: w[p=(i16*8+d), blk, j*16+e] = W[blk*16+i16, j, d, e]
    w = (
        W.reshape(NBLK, 16, NO, DI, DO)          # blk, i16, j, d, e
        .transpose(1, 3, 0, 2, 4)                # i16, d, blk, j, e
        .reshape(128, NBLK, JE)
        .astype(BF16)
    )
    # us[c][p=(i16,d), blk, b] = u[c*8+b, blk*16+i16, d]
    ur = u.reshape(NC_CORES, BL, NBLK, 16, DI)   # c, b, blk, i16, d
    us = np.ascontiguousarray(
        ur.transpose(0, 3, 4, 2, 1)).reshape(NC_CORES, 128, NBLK, BL)
    us = us.astype(BF16)
    wu = np.concatenate(
        [np.broadcast_to(w[None], (NC_CORES,) + w.shape), us], axis=3)
    # diag mask dm[(j8,e), (b',j8')] = (j8 == j8')
    dm = (np.arange(128)[:, None] // 16 == np.arange(64)[None, :] % 8
          ).astype(BF16)
    # bmask bm[p=(b,i16), (b',j8)] = (b == b')
    bm = (np.arange(128)[:, None] // 16 == np.arange(64)[None, :] // 8
          ).astype(BF16)
    # imask mk[p=(i16,d), i16'] = (i16 == i16')
    mk = (np.arange(128)[:, None] // 8 == np.arange(16)[None, :]
          ).astype(BF16)
    idm = np.eye(128, dtype=np.float32)
    return wu, dm, bm, mk, idm


def kernel(u, W):
    from concourse.bass_utils import run_bass_kernel_spmd

    key = "prog"
    if key not in _cache:
        _cache[key] = _build_program()
    nc = _cache[key]

    wu, dm, bm, mk, idm = _host_prep(np.asarray(u, np.float32),
                                     np.asarray(W, np.float32))
    in_maps = [
        {"wu": wu[c], "diagmask": dm, "bmask": bm, "imask": mk, "ident": idm}
        for c in range(NC_CORES)
    ]
    res = run_bass_kernel_spmd(nc, in_maps, list(range(NC_CORES)))
    out = np.concatenate([res.results[c]["v_out"] for c in range(NC_CORES)],
                         axis=0)
    return out.reshape(B, NO, DO).astype(np.float32)
